# revision 22
# baseline (speedup 1.0000x reference)
"""Trainium2 Bass kernel for nn_DiscriminatorModel (8-layer MLP with
LayerNorm+LeakyReLU, 524288x128 input, data-parallel over 8 NeuronCores).

Algorithm (validated vs the jax reference to ~7e-4 relative absmax):
  - Mean-centering of each LayerNorm is folded into the weights host-side:
    Wc_l = W_l @ (I - 11^T/d)  => matmul output is already centered.
  - LayerNorm gammas are folded into the NEXT layer's weights host-side
    (LReLU(g*z) = g*LReLU(z) for g > 0), so the on-device activation is a
    plain LeakyReLU with no per-feature scale.
  - The per-row rsqrt(var+eps) scales commute through LeakyReLU and the
    following matmul; only the layer-6/7 variances matter to fp32 precision:
        E8 = v7 + eps*v6,   out = (a7 @ W8) / sqrt(E8) + b8
  - fp32-grade precision via fp16 multi-word matmuls (3 terms):
        z = Wh@ah + Wh@al + Wl@ah,  fp32 PSUM accumulate.
  - Activations are packed feature-major: 128 partitions = c blocks x dout
    features, rows along the free dim. Col/row tile_position packing keeps
    concurrent matmuls on the PE array.
  - Software pipelining: the serial L4->L7 ladder of supertile k is emitted
    interleaved with the wide L1-L3 matmuls of supertile k+1, so the PE
    never waits on the act chain; input DMA is prefetched 2 supertiles
    ahead. Per layer the activation split is three elementwise passes:
        A: af32 = LReLU(psum) [ScalarE]
        B: ah = fp16(af32)    [VectorE or GpSimd per layer]
        C: al = af32 - ah     [VectorE or GpSimd per layer]

Requires all LayerNorm beta == 0 and gamma > 0 (true for the reference
inputs); otherwise falls back to a numpy forward pass.
"""

import numpy as np

EPS = 1e-5
SLOPE = 0.2
DIMS = [128, 32, 64, 32, 16, 8, 4, 2]
N_CORES = 8
ROWS = 524288
RPC = ROWS // N_CORES        # 65536 rows per core
R_ST = 8192                  # rows per supertile
N_ST = RPC // R_ST           # 8 supertiles per core
F16 = np.float16

_CACHE = {}


def _lrelu(x):
    return np.where(x > 0, x, SLOPE * x).astype(np.float32)


def _center(W):
    d = W.shape[1]
    return (np.asarray(W, np.float64) @ (np.eye(d) - 1.0 / d))


def _split(a):
    hi = a.astype(F16)
    lo = (a.astype(np.float32) - hi.astype(np.float32)).astype(F16)
    return hi, lo


def _blockdiag(W, c):
    din, dout = W.shape
    out = np.zeros((c * din, c * dout), W.dtype)
    for b in range(c):
        out[b * din:(b + 1) * din, b * dout:(b + 1) * dout] = W
    return out


def _transition_stat(W, c_in):
    """Parity-interleaved stationary for a c_in -> 2*c_in packing transition.

    Two stats (par=0,1), each [128, 128]: out col m = blk_out*w + f where
    w = 128/(2*c_in) per-block output width; nonzero iff blk_out % 2 == par,
    source block g = blk_out // 2 maps rows g*din..(g+1)*din <- W[:, f].
    """
    din, dout = W.shape
    w = 128 // (2 * c_in)
    assert w == dout
    stats = []
    for par in range(2):
        S = np.zeros((128, 128), W.dtype)
        for m in range(128):
            blk_out, f = divmod(m, w)
            if blk_out % 2 != par:
                continue
            g = blk_out // 2
            S[g * din:(g + 1) * din, m] = W[:, f]
        stats.append(S)
    return stats


def _var_stats(dout6, dout7):
    # V6 par-stats: s6 is 32-packed (32 blocks x 4 feats); v6' is 64 blocks.
    V6 = []
    for par in range(2):
        S = np.zeros((128, 64), np.float32)
        for m in range(64):
            if m % 2 != par:
                continue
            g = m // 2
            S[g * dout6:(g + 1) * dout6, m] = 1.0 / dout6
        V6.append(S)
    V7 = np.zeros((128, 64), np.float32)
    for m in range(64):
        V7[m * dout7:(m + 1) * dout7, m] = 1.0 / dout7
    return V6[0], V6[1], V7


def _numpy_forward(inp):
    h = np.asarray(inp["x"], np.float32)
    for i in range(7):
        W = np.asarray(inp[f"W{i+1}"], np.float32)
        g = np.asarray(inp[f"g{i+1}"], np.float32)
        b = np.asarray(inp[f"bt{i+1}"], np.float32)
        h = h @ W
        m = h.mean(-1, keepdims=True)
        v = np.square(h - m).mean(-1, keepdims=True)
        h = (h - m) / np.sqrt(v + EPS) * g + b
        h = _lrelu(h)
    return (h @ np.asarray(inp["W8"], np.float32)
            + np.asarray(inp["b8"], np.float32)).astype(np.float32)


def _build_consts(inp):
    """Host-side weight prep (gamma folded into next W). Returns fp16 pack."""
    gs = [np.asarray(inp[f"g{l}"], np.float64) for l in range(1, 8)]
    Ws = [np.asarray(inp[f"W{l}"], np.float64) for l in range(1, 8)]
    # fold gamma_{l-1} into W_l rows; gamma_7 into W8
    Wf = [Ws[0]]
    for i in range(1, 7):
        Wf.append(np.diag(gs[i - 1]) @ Ws[i])
    W8f = (np.diag(gs[6]) @ np.asarray(inp["W8"], np.float64)).astype(np.float32)
    Wc = [_center(Wf[i]).astype(np.float32) for i in range(7)]

    # L1 runs as 3-term fp16 (x arrives as an fp16 hi/lo pair); every later
    # layer runs a single exact-fp32 matmul, so those stationaries are fp32.
    c16 = {}
    h1, l1 = _split(Wc[0])
    c16["s1h"], c16["s1l"] = h1, l1

    c32 = {}
    bd2 = _blockdiag(Wc[1], 2)
    c32["s2"] = np.vstack([bd2, bd2]).astype(np.float32)
    c32["s3"] = _blockdiag(Wc[2], 2).astype(np.float32)
    for l, c_in in ((4, 4), (5, 8), (6, 16), (7, 32)):
        t0, t1 = _transition_stat(Wc[l - 1], c_in)
        c32[f"t{l}a"] = t0.astype(np.float32)
        c32[f"t{l}b"] = t1.astype(np.float32)
    c32["s8"] = _blockdiag(W8f, 64).astype(np.float32)
    V6a, V6b, V7 = _var_stats(DIMS[6], DIMS[7])
    c32["v6a"], c32["v6b"], c32["v7"] = V6a, V6b, V7

    def pack(cols, dt):
        order = sorted(cols.keys())
        offs, total = {}, 0
        for k in order:
            offs[k] = total
            total += cols[k].shape[1]
        arr = np.zeros((128, total), dt)
        for k in order:
            arr[:, offs[k]:offs[k] + cols[k].shape[1]] = cols[k]
        return arr, offs

    wpack, offs16 = pack(c16, F16)
    wpack32, offs32 = pack(c32, np.float32)
    return wpack, offs16, wpack32, offs32


def _split_multi_waits(nc):
    """Walrus build limit: <=1 sync wait per instruction. Hoist extras onto
    same-engine NOPs inserted just before the instruction."""
    import concourse.mybir as mybir
    import bass_rust
    cnt = 0
    for f in nc.m.functions:
        for blk in f.blocks:
            newlist = []
            for inst in blk.instructions:
                si = inst.sync_info
                waits = list(si.on_wait) if si is not None and si.on_wait else []
                if len(waits) > 1:
                    for w in waits[:-1]:
                        nop = mybir.InstNoOp(name=f"waitnop_{cnt}", ins=[], outs=[])
                        cnt += 1
                        nop.engine = inst.engine
                        nop.sync_info = bass_rust.SyncInfo(on_wait=[w], on_update=[])
                        newlist.append(nop)
                    inst.sync_info = bass_rust.SyncInfo(
                        on_wait=[waits[-1]], on_update=list(si.on_update))
                newlist.append(inst)
            blk.instructions = newlist
    return cnt


def _build_program(offs16, w16_cols, offs32, w32_cols, b8_val):
    import concourse.bass as bass
    import concourse.mybir as mybir
    from concourse.tile import TileContext
    from contextlib import ExitStack

    # this walrus build rejects >1 sync wait on the tail Drain; split them
    import bass_rust
    from concourse.tile import TileContext as _TC
    from concourse.vector_clock import ScopedClock

    def _patched_drain(self, tick_clock, wait_clock):
        probe = self.nc.sync.nop()
        wait_clock.add_sem_waits(probe.ins,
                                 ScopedClock({None: tick_clock.global_clock}))
        si = probe.ins.sync_info
        waits = list(si.on_wait) if si is not None else []
        upd = list(si.on_update) if si is not None else []
        probe.ins.sync_info = bass_rust.SyncInfo(on_wait=waits[:1], on_update=upd)
        for w in waits[1:]:
            nop = self.nc.sync.nop()
            nop.ins.sync_info = bass_rust.SyncInfo(on_wait=[w], on_update=[])
        self.nc.sync.drain()
        self.nc.all_engine_barrier()
        assert self.sems is not None
        popped = self.nc._tile_sem_poison_stack.pop()
        assert popped is self._sem_poison
        self.nc.clear_and_free_semaphores(list(self.sems.allocated().values()))
        self.nc.all_engine_barrier()

    _TC._drain_and_barrier = _patched_drain

    f16, f32 = mybir.dt.float16, mybir.dt.float32
    AF = mybir.ActivationFunctionType
    OP = mybir.AluOpType

    nc = bass.Bass(trn_type="TRN2", num_swdge_queues=4)
    xhi_d = nc.dram_tensor("xhi", [128, RPC], f16, kind="ExternalInput")
    xlo_d = nc.dram_tensor("xlo", [128, RPC], f16, kind="ExternalInput")
    wp_d = nc.dram_tensor("wpack", [128, w16_cols], f16, kind="ExternalInput")
    wp32_d = nc.dram_tensor("wpack32", [128, w32_cols], f32,
                            kind="ExternalInput")
    out_d = nc.dram_tensor("out", [N_ST * 64, R_ST // 64], f32,
                           kind="ExternalOutput")

    with TileContext(nc) as tc:
        with ExitStack() as ctx:
            const = ctx.enter_context(tc.tile_pool(name="const", bufs=1))
            wp = const.tile([128, w16_cols], f16)
            nc.sync.dma_start(wp[:, :], wp_d[:, :])
            wp32 = const.tile([128, w32_cols], f32)
            nc.sync.dma_start(wp32[:, :], wp32_d[:, :])

            def W16(name):
                return wp[:, offs16[name]:offs16[name] + _WCOLS16[name]]

            def W32(name):
                return wp32[:, offs32[name]:offs32[name] + _WCOLS32[name]]

            xp = ctx.enter_context(tc.tile_pool(name="xp", bufs=2))
            ap = ctx.enter_context(tc.tile_pool(name="ap", bufs=2))
            fin = ctx.enter_context(tc.tile_pool(name="fin", bufs=3))
            up = ctx.enter_context(tc.tile_pool(name="up", bufs=3, space="PSUM"))
            vp = ctx.enter_context(tc.tile_pool(name="vp", bufs=2, space="PSUM"))

            def mm(out, lhsT, rhs, start, stop, tp=None):
                # matmul output must fit one PSUM bank: 512 fp32 columns
                n = out.shape[1]
                for o in range(0, n, 512):
                    e = min(o + 512, n)
                    nc.tensor.matmul(out[:, o:e], lhsT, rhs[:, o:e],
                                     start=start, stop=stop, tile_position=tp)

            def mm_multi(parts, start, stop):
                """Emit chunk-outer / tile-inner so adjacent instructions hit
                disjoint PE subarrays (avoids FIFO head-of-line blocking)."""
                n = parts[0][0].shape[1]
                for o in range(0, n, 512):
                    e = min(o + 512, n)
                    for out, lhsT, rhs, tp in parts:
                        nc.tensor.matmul(out[:, o:e], lhsT, rhs[:, o:e],
                                         start=start, stop=stop,
                                         tile_position=tp)

            ysbs, e8sbs = [], []
            dma_engs = [nc.sync, nc.gpsimd, nc.scalar, nc.gpsimd]
            xtiles = {}

            def emit_dma(st, pieces=1):
                # pieces>1 splits each half-tile DMA so the first L1 matmuls
                # can start as soon as their slice lands (startup latency)
                x0 = st * R_ST
                xh, xl = [], []
                w = 4096 // pieces
                for k in range(2):
                    xht = xp.tile([128, 4096], f16, name=f"xh{k}")
                    for p in range(pieces):
                        dma_engs[k].dma_start(
                            xht[:, p * w:(p + 1) * w],
                            xhi_d[:, x0 + 4096 * k + p * w:
                                  x0 + 4096 * k + (p + 1) * w])
                    xh.append(xht)
                    xlt = xp.tile([128, 4096], f16, name=f"xl{k}")
                    for p in range(pieces):
                        dma_engs[2 + k].dma_start(
                            xlt[:, p * w:(p + 1) * w],
                            xlo_d[:, x0 + 4096 * k + p * w:
                                  x0 + 4096 * k + (p + 1) * w])
                    xl.append(xlt)
                xtiles[st] = (xh, xl)

            def act(u, n, dst, col0, eng=None):
                """Single LeakyReLU pass: PSUM fp32 -> SBUF fp32 act tile."""
                (eng or nc.scalar).activation(
                    dst[:, col0:col0 + n], u[:, :n], AF.Prelu,
                    bias=0.0, scale=1.0, alpha=SLOPE)

            SA = {}  # per-supertile phase-A state
            SB = {}  # per-supertile phase-B (ladder) state

            s2s = [W32("s2")[64 * q:64 * (q + 1), :] for q in range(2)]

            def emit_A(st, i):
                s = SA.setdefault(st, {})
                if i == 0:
                    s["a1"] = ap.tile([128, 2048], f32, name="a1")
                if i in (0, 1):
                    # ---- L1 chunk c=i: fp16 3-term, col-tiled 4x [128,32]
                    c = i
                    xh, xl = xtiles[st]
                    u = up.tile([128, 1024], f32, name="u", tag="u")
                    for t in range(3):
                        S = W16("s1h") if t < 2 else W16("s1l")
                        parts = []
                        for b in range(4):
                            xsrc = xh if t != 1 else xl
                            r = xsrc[b // 2][:, (b % 2) * 2048 + 1024 * c:][:, :1024]
                            parts.append((u[32 * b:32 * (b + 1), :], S, r,
                                          (0, 32 * b)))
                        mm_multi(parts, start=(t == 0), stop=(t == 2))
                    act(u, 1024, s["a1"], 1024 * c)
                    if i == 1:
                        # all readers of this supertile's x tiles are emitted;
                        # prefetch the st+2 input into the freed xp buffers
                        del xtiles[st]
                        if st + 2 < N_ST:
                            emit_dma(st + 2)
                if i == 2:
                    s["a2"] = [ap.tile([128, 2048], f32, name=f"a2q{q}")
                               for q in range(2)]
                if i in (2, 3):
                    # ---- L2 chunk c=i-2: fp32, row-tiled 2x [64,128]
                    c = i - 2
                    us = [up.tile([128, 1024], f32, name="u", tag="u")
                          for _ in range(2)]
                    mm_multi([(us[q][:, :], s2s[q],
                               s["a1"][64 * q:64 * (q + 1),
                                       1024 * c:1024 * (c + 1)],
                               (64 * q, 0)) for q in range(2)],
                             start=True, stop=True)
                    for q in range(2):
                        act(us[q], 1024, s["a2"][q], 1024 * c)
                if i == 4:
                    s["a3"] = ap.tile([128, 2048], f32, name="a3")
                if i in (4, 5):
                    # ---- L3 chunk c=i-4: fp32, col-tiled 2x [128,64]
                    c = i - 4
                    u = up.tile([128, 1024], f32, name="u", tag="u")
                    mm_multi([(u[64 * q:64 * (q + 1), :], W32("s3"),
                               s["a2"][q][:, 1024 * c:1024 * (c + 1)],
                               (0, 64 * q)) for q in range(2)],
                             start=True, stop=True)
                    act(u, 1024, s["a3"], 1024 * c)

            def emit_B(st, i):
                s = SA[st]
                b = SB.setdefault(st, {})
                if i == 0:
                    b["prev"], b["n"] = s["a3"], 2048
                if i < 4:
                    # ---- L4+i: fp32 parity transition, halving free size
                    l = 4 + i
                    n = b["n"] // 2
                    u = up.tile([128, 1024], f32, name="u", tag="u")
                    uv = u[:, :n]
                    for par, suf in ((0, "a"), (1, "b")):
                        mm(uv, W32(f"t{l}{suf}"),
                           b["prev"][:, par * n:(par + 1) * n],
                           start=(par == 0), stop=(par == 1))
                    na = ap.tile([128, n], f32, name=f"a{l}")
                    if l == 6:
                        # fp32 squares (fp16 would underflow degenerate rows)
                        b["s6"] = ap.tile([128, 256], f32, name="s6")
                        nc.scalar.activation(b["s6"][:, :], uv, AF.Square)
                    if l == 7:
                        b["s7"] = ap.tile([128, 128], f32, name="s7")
                        nc.scalar.activation(b["s7"][:, :], uv, AF.Square)
                    act(u, n, na, 0)
                    b["prev"], b["n"] = na, n
                    return
                # ---- i == 4: tail. L8 first (no deps on variances).
                yt = vp.tile([64, 128], f32, name="yt", tag="v")
                mm(yt[:, :], W32("s8"), b["prev"][:, :], start=True, stop=True)
                ysb = fin.tile([64, 128], f32, name="ysb", tag="ysb")
                nc.scalar.copy(ysb[:, :], yt[:, :])
                v6t = vp.tile([64, 128], f32, name="v6t", tag="v")
                mm(v6t[:, :], W32("v6a"), b["s6"][:, 0:128],
                   start=True, stop=False)
                mm(v6t[:, :], W32("v6b"), b["s6"][:, 128:256],
                   start=False, stop=True)
                v7t = vp.tile([64, 128], f32, name="v7t", tag="v")
                mm(v7t[:, :], W32("v7"), b["s7"][:, :],
                   start=True, stop=True)
                v7sb = fin.tile([64, 128], f32, name="v7sb", tag="v7sb", bufs=2)
                nc.scalar.copy(v7sb[:, :], v7t[:, :])
                e8 = fin.tile([64, 128], f32, name="e8", tag="e8")
                nc.vector.scalar_tensor_tensor(e8[:, :], v6t[:, :], EPS,
                                               v7sb[:, :], OP.mult, OP.add)
                SA.pop(st)
                SB.pop(st)
                # final: out = y / sqrt(E8) + b8, spread across supertiles
                sq = fin.tile([64, 128], f32, name="sq", tag="sq", bufs=2)
                nc.scalar.activation(sq[:, :], e8[:, :], AF.Sqrt)
                rinv = fin.tile([64, 128], f32, name="rinv", tag="rinv",
                                bufs=2)
                nc.vector.reciprocal(rinv[:, :], sq[:, :])
                osb = fin.tile([64, 128], f32, name="osb", tag="osb", bufs=2)
                nc.vector.tensor_tensor(osb[:, :], ysb[:, :], rinv[:, :],
                                        OP.mult)
                nc.vector.tensor_scalar(osb[:, :], osb[:, :], b8_val,
                                        None, OP.add)
                nc.sync.dma_start(out_d[st * 64:(st + 1) * 64, :], osb[:, :])

            emit_dma(0, pieces=4)
            emit_dma(1, pieces=2)
            for st in range(N_ST + 1):
                for i in range(6):
                    if st < N_ST:
                        emit_A(st, i)
                    if st >= 1 and i < 5:
                        emit_B(st - 1, i)

    _split_multi_waits(nc)
    return nc


_WCOLS16 = {}
_WCOLS32 = {}


def kernel(**inputs):
    for l in range(1, 8):
        if np.abs(np.asarray(inputs[f"bt{l}"], np.float32)).max() > 0:
            return _numpy_forward(inputs)
        if np.asarray(inputs[f"g{l}"], np.float32).min() <= 0:
            return _numpy_forward(inputs)

    wpack, offs16, wpack32, offs32 = _build_consts(inputs)
    global _WCOLS16, _WCOLS32
    _WCOLS16 = {"s1h": 32, "s1l": 32}
    _WCOLS32 = {"s2": 128, "s3": 64, "s8": 64,
                "v6a": 64, "v6b": 64, "v7": 64}
    for l in range(4, 8):
        for suf in ("a", "b"):
            _WCOLS32[f"t{l}{suf}"] = 128

    x = np.asarray(inputs["x"], np.float32)
    xT = np.ascontiguousarray(x.T)               # [128, 524288]
    xhi = xT.astype(F16)
    xlo = (xT - xhi.astype(np.float32)).astype(F16)
    b8 = np.asarray(inputs["b8"], np.float32).reshape(1, 1)

    nc = _build_program(offs16, wpack.shape[1], offs32, wpack32.shape[1],
                        float(b8[0, 0]))

    in_maps = []
    for c in range(N_CORES):
        s = slice(c * RPC, (c + 1) * RPC)
        in_maps.append({
            "xhi": np.ascontiguousarray(xhi[:, s]),
            "xlo": np.ascontiguousarray(xlo[:, s]),
            "wpack": wpack, "wpack32": wpack32,
        })

    from concourse.bass_utils import run_bass_kernel_spmd
    res = run_bass_kernel_spmd(nc, in_maps, core_ids=list(range(N_CORES)))

    out = np.empty((ROWS, 1), np.float32)
    for c in range(N_CORES):
        out[c * RPC:(c + 1) * RPC, 0] = res.results[c]["out"].reshape(-1)
    return out
